# revision 21
# baseline (speedup 1.0000x reference)
"""Trainium2 Bass kernel for nn_NegSimHead (loss_fn).

Reference computation (N=8192, C=512):
  v = normalize(v_feat); t = normalize(t_feat); pv = normalize(p_v); pt = normalize(p_t)
  neg_sim = -0.5*mean(sum(pv*t,1)) - 0.5*mean(sum(pt*v,1))
  stats(x) = mean(std(x, axis=0, ddof=1)) for each normalized tensor
  s1 = v @ pt.T ; s2 = t @ pv.T
  retrieval(s): pos[i] = rank of s[i,i] in row i (descending) = #{j: s[i,j] > s[i,i]}
  out [13] = [neg_sim, stats(v), stats(t), stats(pv), stats(pt),
              r1,r5,r10,mr of s1, r1,r5,r10,mr of s2]

Strategy (8 cores, data-parallel over N):
  Core k gets rows k*1024..(k+1)*1024 of v/t (transposed, [512,1024]) and the FULL
  p_t/p_v transposed and ROLLED by -k*1024 rows, so that local column j of the
  similarity strip corresponds to global row (k*1024+j)%N.  The diagonal of the
  full similarity matrix then sits at static local positions (strip mb's diagonal
  is in column block mb) identically on every core -> pure SPMD program.

  Row-normalization of v/t scales whole rows of s and cancels in the rank
  comparison, so raw vT/tT feed the matmul directly.  p̂T is column-normalized on
  device (squares -> all-ones matmul partition-reduce -> reciprocal -> sqrt ->
  scale).  Matmuls run in float32r (fp22 mantissa, full PE speed at free dim 512).
  The diagonal d is extracted bit-exactly from the matmul output (identity mask
  multiply + reduce), so the self-comparison contributes exactly zero.  Counting
  is split between ScalarE (Sign(s-d) with per-partition bias, fused accumulate)
  and VectorE (is_gt with per-partition scalar, fused accumulate).

  Each core reduces its partials to a single [128, 88] tile (stats sums, loss
  sums, and per-partition retrieval counts/sums computed from pos on device),
  which is AllReduce-summed across the 8 cores.  The host fetches ONE core's
  shard -- a single tunnel round trip, which dominates the measured execute
  time -- and finishes the ~0.1KB of scalar math.
"""
import threading
import time
import numpy as np
from contextlib import ExitStack

import concourse.bacc as bacc
import concourse.tile as tile
from concourse import mybir

F32 = mybir.dt.float32
F32R = mybir.dt.float32r
ALU = mybir.AluOpType
AX = mybir.AxisListType
AF = mybir.ActivationFunctionType

N = 8192          # batch
C = 512           # feature dim
NCORES = 8
S = N // NCORES   # rows per core = 1024
KC = C // 128     # contraction chunks = 4
MB = S // 128     # row strips per core = 8
NTILE = 512       # similarity column tile
NT = N // NTILE   # column tiles = 16
# column tiles counted on ScalarE (Sign) vs VectorE (is_gt).  Diagonal tiles
# (nt 0,1) must be on the DVE/is_gt path so exact ties (the diagonal) count 0.
ACT_SET = frozenset(range(2, 10))
C_ACT = len(ACT_SET) * NTILE

_CACHE = {}
RESULTS = None  # last BassKernelResults (exec_time_ns etc.), for test harnesses
# Background tiny-put traffic during the blocking fetch.  Helps bare fetches
# of already-computed data (~85 -> ~33 ms) but measured indistinguishable for
# the execute+fetch path, so off by default.
CHURN = False


def _build_program():
    nc = bacc.Bacc("TRN2", target_bir_lowering=False, debug=False,
                   num_devices=NCORES)

    xT_d = [nc.dram_tensor("vT", [C, S], F32R, kind="ExternalInput").ap(),
            nc.dram_tensor("tT", [C, S], F32R, kind="ExternalInput").ap()]
    P_d = [nc.dram_tensor("ptT", [C, N], F32R, kind="ExternalInput").ap(),
           nc.dram_tensor("pvT", [C, N], F32R, kind="ExternalInput").ap()]
    ident_d = nc.dram_tensor("ident", [128, 128], F32, kind="ExternalInput").ap()
    ones_d = nc.dram_tensor("ones", [128, 128], F32R, kind="ExternalInput").ap()

    # single output, AllReduce-summed over cores:
    #   cols 0..63  stats  [tensor(4: v,t,pv,pt)][k(4)][half(2)][sum/sumsq(2)]
    #   cols 64..79 loss   [phase(2)][k(4)][half(2)]
    #   cols 80..87 retrieval partials [phase(2)][lt1,lt5,lt10,possum]
    o_all_d = nc.dram_tensor("o_all", [128, 88], F32, kind="ExternalOutput").ap()

    with tile.TileContext(nc) as tc, ExitStack() as ctx:
        persist = ctx.enter_context(tc.tile_pool(name="persist", bufs=1))
        ppool = ctx.enter_context(tc.tile_pool(name="ppool", bufs=1))
        sq_pool = ctx.enter_context(tc.tile_pool(name="sq", bufs=2))
        b_pool = ctx.enter_context(tc.tile_pool(name="bb", bufs=2))
        xh_pool = ctx.enter_context(tc.tile_pool(name="xh", bufs=2))
        scr_pool = ctx.enter_context(tc.tile_pool(name="scr", bufs=2))
        mm_psum = ctx.enter_context(tc.tile_pool(name="mmps", bufs=7, space="PSUM"))
        nrm_psum = ctx.enter_context(tc.tile_pool(name="nrmps", bufs=1, space="PSUM"))

        i_t = persist.tile([128, 128], F32, name="i_t")
        ones_t = persist.tile([128, 128], F32R, name="ones_t")
        nc.sync.dma_start(out=i_t, in_=ident_d)
        nc.sync.dma_start(out=ones_t, in_=ones_d)

        o_sgn = persist.tile([128, 2 * MB], F32, name="o_sgn")
        o_cnt = persist.tile([128, 2 * MB], F32, name="o_cnt")
        o_all = persist.tile([128, 88], F32, name="o_all")
        nc.vector.memset(o_all, 0.0)
        LOSS0, MET0 = 64, 80  # column offsets of loss / retrieval sections

        # x shards stay resident for the whole kernel
        xT = [[persist.tile([128, S], F32R, name=f"xT{ph}_{k}")
               for k in range(KC)] for ph in range(2)]

        # persistent per-phase state
        P = [[[None] * NT for _ in range(KC)] for _ in range(2)]
        invb_x = [persist.tile([128, S], F32, name=f"invb_x{ph2}")
                  for ph2 in range(2)]
        d_sb = [persist.tile([128, MB], F32, name=f"d{ph2}") for ph2 in range(2)]
        negd_sb = [persist.tile([128, MB], F32, name=f"negd{ph2}")
                   for ph2 in range(2)]
        cnts = [persist.tile([128, MB, NT], F32, name=f"cnts{ph2}")
                for ph2 in range(2)]
        sgns = [persist.tile([128, MB, NT], F32, name=f"sgns{ph2}")
                for ph2 in range(2)]
        for ph in range(2):
            nc.vector.memset(cnts[ph], 0.0)
            nc.vector.memset(sgns[ph], 0.0)

        def x_norm(ph):
            for h in range(2):
                hs = slice(h * 512, (h + 1) * 512)
                ps_x = nrm_psum.tile([128, 512], F32, name=f"psx{ph}_{h}",
                                     tag="nrm")
                for k in range(KC):
                    sqx = sq_pool.tile([128, 512], F32R,
                                       name=f"sqx{ph}_{k}_{h}", tag="sqx")
                    nc.scalar.square(sqx, xT[ph][k][:, hs])
                    nc.tensor.matmul(ps_x, ones_t, sqx,
                                     start=(k == 0), stop=(k == KC - 1))
                nc.vector.reciprocal(invb_x[ph][:, hs], ps_x)
                nc.scalar.sqrt(invb_x[ph][:, hs], invb_x[ph][:, hs])

        def load_and_norm_P(ph, nt):
            # DMA the 4 contraction chunks of column tile nt, then column-
            # normalize in place (squares -> all-ones matmul partition-sum ->
            # rsqrt -> scale).  Tags are shared across phases (bufs=1), so
            # phase 1's DMA naturally waits for phase 0's last reader.
            for k in range(KC):
                pt_ = ppool.tile([128, NTILE], F32R, name=f"P_{k}_{nt}",
                                 tag=f"P_{k}_{nt}")
                nc.sync.dma_start(
                    out=pt_, in_=P_d[ph][k * 128:(k + 1) * 128,
                                         nt * NTILE:(nt + 1) * NTILE])
                P[ph][k][nt] = pt_
            ps_n = nrm_psum.tile([128, NTILE], F32, name=f"psn{ph}_{nt}",
                                 tag="nrm")
            for k in range(KC):
                sq = sq_pool.tile([128, NTILE], F32R, name=f"sq{ph}_{nt}_{k}",
                                  tag="sq")
                nc.scalar.square(sq, P[ph][k][nt])
                nc.tensor.matmul(ps_n, ones_t, sq,
                                 start=(k == 0), stop=(k == KC - 1))
            b_t = b_pool.tile([128, NTILE], F32, name=f"b{ph}_{nt}", tag="b")
            nc.vector.reciprocal(b_t, ps_n)
            nc.scalar.sqrt(b_t, b_t)
            for k in range(KC):
                nc.vector.tensor_mul(P[ph][k][nt], P[ph][k][nt], b_t)

        def stats_chunk(ph, k):
            # stats tensor index: x side: v(0), t(1); P side: pt(3), pv(2)
            pstat = 3 if ph == 0 else 2
            for h in range(2):
                col = pstat * 16 + k * 4 + h * 2
                nc.vector.tensor_reduce(o_all[:, col:col + 1],
                                        P[ph][k][h], axis=AX.X, op=ALU.add)
                pscr = scr_pool.tile([128, NTILE], F32,
                                     name=f"pscr{ph}_{k}_{h}", tag="scr")
                nc.scalar.activation(out=pscr, in_=P[ph][k][h],
                                     func=AF.Square,
                                     accum_out=o_all[:, col + 1:col + 2])
            xh = xh_pool.tile([128, S], F32, name=f"xh{ph}_{k}", tag="xh")
            nc.vector.tensor_mul(xh, xT[ph][k], invb_x[ph])
            for h in range(2):
                col = ph * 16 + k * 4 + h * 2
                hs = slice(h * 512, (h + 1) * 512)
                nc.vector.tensor_reduce(o_all[:, col:col + 1], xh[:, hs],
                                        axis=AX.X, op=ALU.add)
                xscr = scr_pool.tile([128, 512], F32,
                                     name=f"xscr{ph}_{k}_{h}", tag="scr")
                nc.scalar.activation(out=xscr, in_=xh[:, hs],
                                     func=AF.Square,
                                     accum_out=o_all[:, col + 1:col + 2])
                # loss: sum(x-hat * p-hat) over own shard rows
                lscr = scr_pool.tile([128, 512], F32,
                                     name=f"lscr{ph}_{k}_{h}", tag="scr")
                nc.vector.tensor_mul(lscr, xh[:, hs], P[ph][k][h])
                lcol = LOSS0 + ph * 8 + k * 2 + h
                nc.vector.tensor_reduce(o_all[:, lcol:lcol + 1], lscr,
                                        axis=AX.X, op=ALU.add)

        def mm_strip(ph, mb, nt):
            ps = mm_psum.tile([128, NTILE], F32, name=f"ps{ph}_{mb}_{nt}",
                              tag="mm")
            for k in range(KC):
                nc.tensor.matmul(ps, xT[ph][k][:, mb * 128:(mb + 1) * 128],
                                 P[ph][k][nt], start=(k == 0),
                                 stop=(k == KC - 1))
            return ps

        def d_pass(ph):
            # for each strip, compute its diagonal-containing tile first,
            # extract d (bit-exact: identity-mask multiply + reduce), and
            # count that tile on the DVE/is_gt path (self-comparison = 0)
            for mb in range(MB):
                nt_d = (mb * 128) // NTILE
                ps = mm_strip(ph, mb, nt_d)
                sub = (mb * 128) % NTILE
                dscr = scr_pool.tile([128, 128], F32, name=f"dscr{ph}_{mb}",
                                     tag="dscr")
                nc.vector.tensor_mul(dscr, ps[:, sub:sub + 128], i_t)
                nc.vector.tensor_reduce(d_sb[ph][:, mb:mb + 1], dscr,
                                        axis=AX.X, op=ALU.add)
                nc.vector.tensor_scalar_mul(negd_sb[ph][:, mb:mb + 1],
                                            d_sb[ph][:, mb:mb + 1], -1.0)
                cscr = scr_pool.tile([128, NTILE], F32, name=f"cscr{ph}_{mb}",
                                     tag="cscr")
                nc.vector.tensor_scalar(
                    out=cscr, in0=ps, scalar1=d_sb[ph][:, mb:mb + 1],
                    scalar2=0.0, op0=ALU.is_gt, op1=ALU.add,
                    accum_out=cnts[ph][:, mb, nt_d:nt_d + 1])

        def main_col(ph, nt):
            for mb in range(MB):
                if nt == (mb * 128) // NTILE:
                    continue  # handled in the d-pass
                ps = mm_strip(ph, mb, nt)
                if nt in ACT_SET:
                    ascr = scr_pool.tile([128, NTILE], F32,
                                         name=f"a{ph}_{nt}_{mb}", tag="ascr")
                    nc.scalar.activation(
                        out=ascr, in_=ps, func=AF.Sign,
                        bias=negd_sb[ph][:, mb:mb + 1], scale=1.0,
                        accum_out=sgns[ph][:, mb, nt:nt + 1])
                else:
                    cscr = scr_pool.tile([128, NTILE], F32,
                                         name=f"c{ph}_{nt}_{mb}", tag="cscr")
                    nc.vector.tensor_scalar(
                        out=cscr, in0=ps, scalar1=d_sb[ph][:, mb:mb + 1],
                        scalar2=0.0, op0=ALU.is_gt, op1=ALU.add,
                        accum_out=cnts[ph][:, mb, nt:nt + 1])

        def reduce_slots(ph):
            for mb in range(MB):
                c = ph * MB + mb
                nc.vector.tensor_reduce(o_cnt[:, c:c + 1], cnts[ph][:, mb, :],
                                        axis=AX.X, op=ALU.add)
                nc.vector.tensor_reduce(o_sgn[:, c:c + 1], sgns[ph][:, mb, :],
                                        axis=AX.X, op=ALU.add)
            # pos[p, mb] = cnt + (sgn + C_ACT)/2, an exact small integer.
            # Reduce to per-partition partials: counts of pos<{1,5,10} and
            # sum(pos); summed across cores by the final AllReduce.
            sl = slice(ph * MB, (ph + 1) * MB)
            pos = scr_pool.tile([128, MB], F32, name=f"pos{ph}", tag="pos")
            nc.vector.tensor_scalar(out=pos, in0=o_sgn[:, sl],
                                    scalar1=float(C_ACT), scalar2=0.5,
                                    op0=ALU.add, op1=ALU.mult)
            nc.vector.tensor_add(pos, pos, o_cnt[:, sl])
            for j, thr in enumerate((1.0, 5.0, 10.0)):
                tscr = scr_pool.tile([128, MB], F32, name=f"lt{ph}_{j}",
                                     tag="pos")
                c = MET0 + ph * 4 + j
                nc.vector.tensor_scalar(out=tscr, in0=pos, scalar1=thr,
                                        scalar2=0.0, op0=ALU.is_lt,
                                        op1=ALU.add,
                                        accum_out=o_all[:, c:c + 1])
            c = MET0 + ph * 4 + 3
            nc.vector.tensor_reduce(o_all[:, c:c + 1], pos, axis=AX.X,
                                    op=ALU.add)

        # ---- emission order (Tile priority / engine-FIFO order follows
        # program order, so interleave cross-phase work deliberately):
        # the P-column load+normalize stream leads the matmul+count stream by
        # two columns, and phase 1's loads trail phase 0's last reader. ----
        load_and_norm_P(0, 0)
        for k in range(KC):
            nc.sync.dma_start(out=xT[0][k],
                              in_=xT_d[0][k * 128:(k + 1) * 128, :])
        load_and_norm_P(0, 1)
        x_norm(0)
        d_pass(0)
        for nt in range(2, NT):
            load_and_norm_P(0, nt)
            m = nt - 2
            main_col(0, m)
            if m < KC:
                stats_chunk(0, m)
            if m == KC:
                for k in range(KC):
                    nc.sync.dma_start(out=xT[1][k],
                                      in_=xT_d[1][k * 128:(k + 1) * 128, :])
                x_norm(1)
            if m >= 5:
                load_and_norm_P(1, m - 5)
        main_col(0, NT - 2)
        load_and_norm_P(1, 9)
        main_col(0, NT - 1)
        load_and_norm_P(1, 10)
        for j in range(11, NT):
            load_and_norm_P(1, j)
        reduce_slots(0)
        d_pass(1)
        for nt in range(NT):
            main_col(1, nt)
            if nt < KC:
                stats_chunk(1, nt)
        reduce_slots(1)

        # Sum the per-core partials across all 8 cores on-device, so the host
        # needs to fetch only ONE core's shard (one tunnel round trip).
        dram = ctx.enter_context(tc.tile_pool(name="dram", bufs=1,
                                              space="DRAM"))
        in_b = dram.tile([128, 88], F32, name="ar_in")
        out_b = dram.tile([128, 88], F32, name="ar_out")
        nc.gpsimd.dma_start(in_b[:], o_all)
        nc.gpsimd.collective_compute(
            "AllReduce", ALU.add,
            replica_groups=[list(range(NCORES))],
            ins=[in_b.opt()], outs=[out_b.opt()])
        nc.gpsimd.dma_start(o_all_d, out_b[:])

    nc.compile()
    return nc


def _get_runner():
    """Build (once) a jitted 8-core SPMD executor for the Bass program.

    Mirrors bass2jax.run_bass_via_pjrt's multi-core branch, but keeps the
    jitted function and pre-staged device inputs so repeated calls skip
    retracing/recompiling, and so transfer vs execute can be timed apart.
    """
    if "runner" in _CACHE:
        return _CACHE["runner"]

    import jax
    import jax.numpy as jnp
    from jax.experimental.shard_map import shard_map
    from jax.sharding import Mesh, PartitionSpec, NamedSharding
    from concourse import mybir as _mybir
    from concourse.bass2jax import (_bass_exec_p, install_neuronx_cc_hook,
                                    partition_id_tensor)

    nc = _CACHE["nc"]
    install_neuronx_cc_hook()

    partition_name = (nc.partition_id_tensor.name
                      if nc.partition_id_tensor else None)
    in_names, out_names, out_avals = [], [], []
    zero_outs = []
    for alloc in nc.m.functions[0].allocations:
        if not isinstance(alloc, _mybir.MemoryLocationSet):
            continue
        name = alloc.memorylocations[0].name
        if alloc.kind == "ExternalInput":
            if name != partition_name:
                in_names.append(name)
        elif alloc.kind == "ExternalOutput":
            out_names.append(name)
            shape = tuple(alloc.tensor_shape)
            dtype = _mybir.dt.np(alloc.dtype)
            out_avals.append(jax.core.ShapedArray(shape, dtype))
            zero_outs.append(np.zeros(shape, dtype))
    n_params = len(in_names)
    all_in_names = in_names + out_names
    if partition_name is not None:
        all_in_names = all_in_names + [partition_name]

    def _body(*args):
        operands = list(args)
        if partition_name is not None:
            operands.append(partition_id_tensor())
        outs = _bass_exec_p.bind(
            *operands,
            out_avals=tuple(out_avals),
            in_names=tuple(all_in_names),
            out_names=tuple(out_names),
            lowering_input_output_aliases=(),
            sim_require_finite=True,
            sim_require_nnan=True,
            nc=nc,
        )
        return tuple(outs)

    devices = jax.devices()[:NCORES]
    mesh = Mesh(np.asarray(devices), ("core",))
    spec = NamedSharding(mesh, PartitionSpec("core"))
    donate = tuple(range(n_params, n_params + len(out_names)))
    sharded = jax.jit(
        shard_map(_body, mesh=mesh,
                  in_specs=(PartitionSpec("core"),) * (n_params + len(out_names)),
                  out_specs=(PartitionSpec("core"),) * len(out_names),
                  check_rep=False),
        donate_argnums=donate, keep_unused=True)

    def run(in_maps):
        t0 = time.time()
        concat_in = [
            np.concatenate([in_maps[c][name] for c in range(NCORES)], axis=0)
            for name in in_names
        ]
        dev_in = [jax.device_put(a, spec) for a in concat_in]
        dev_zero = [jax.device_put(
            np.zeros((NCORES * z.shape[0], *z.shape[1:]), z.dtype), spec)
            for z in zero_outs]
        for a in dev_in + dev_zero:
            a.block_until_ready()
        t1 = time.time()
        # The axon tunnel batches/delays responses when the link is idle
        # (~85 ms for a lone fetch vs ~33 ms when traffic is flowing), so
        # keep a trickle of tiny puts going while we wait for the result.
        stop_churn = threading.Event()
        churn_pad = np.zeros((1, 1), np.float32)

        def _churn():
            dev = devices[0]
            while not stop_churn.is_set():
                try:
                    jax.device_put(churn_pad, dev)
                except Exception:
                    return
                time.sleep(0.001)

        churn_t = None
        if CHURN:
            churn_t = threading.Thread(target=_churn, daemon=True)
            churn_t.start()
        try:
            out_arrs = sharded(*dev_in, *dev_zero)
            # the kernel AllReduce-sums o_all across cores, so every shard
            # holds the same totals: fetch core 0's shard only (one round
            # trip on the tunnel; racing parallel per-device fetches was
            # tried and made it worse -- concurrent fetches contend)
            shard0 = np.asarray(out_arrs[0].addressable_shards[0].data)
        finally:
            stop_churn.set()
            if churn_t is not None:
                churn_t.join()
        t2 = time.time()
        TIMES.update(transfer_s=t1 - t0, execute_s=t2 - t1)
        return shard0

    _CACHE["runner"] = run
    return run


TIMES = {}


def kernel(v_feat, t_feat, p_v, p_t):
    if "nc" not in _CACHE:
        _CACHE["nc"] = _build_program()

    t0 = time.time()
    v = np.ascontiguousarray(v_feat, dtype=np.float32)
    t = np.ascontiguousarray(t_feat, dtype=np.float32)
    pv = np.ascontiguousarray(p_v, dtype=np.float32)
    pt = np.ascontiguousarray(p_t, dtype=np.float32)

    ident = np.eye(128, dtype=np.float32)
    ones = np.ones((128, 128), dtype=np.float32)

    in_maps = []
    for k in range(NCORES):
        sl = slice(k * S, (k + 1) * S)
        in_maps.append({
            "vT": np.ascontiguousarray(v[sl].T),
            "tT": np.ascontiguousarray(t[sl].T),
            "ptT": np.ascontiguousarray(np.roll(pt, -k * S, axis=0).T),
            "pvT": np.ascontiguousarray(np.roll(pv, -k * S, axis=0).T),
            "ident": ident,
            "ones": ones,
        })
    TIMES["prep_s"] = time.time() - t0

    arr = _get_runner()(in_maps).astype(np.float64)  # [128, 88] core-summed

    # retrieval partials (cols 80..87): per-partition counts/sums, exact ints
    def retrieval(ph):
        base = 80 + ph * 4
        lt1 = arr[:, base + 0].sum()
        lt5 = arr[:, base + 1].sum()
        lt10 = arr[:, base + 2].sum()
        psum = arr[:, base + 3].sum()
        return lt1 / N, lt5 / N, lt10 / N, psum / N

    v_r1, v_r5, v_r10, v_mr = retrieval(0)
    t_r1, t_r5, t_r10, t_mr = retrieval(1)

    # stats (cols 0..63): std per feature column (ddof=1), mean over columns
    st = arr[:, 0:64].reshape(128, 4, KC, 2, 2)  # [p, tensor, k, h, sum/sumsq]
    ssum = st[..., 0].sum(axis=3)                # [p, tensor, k]
    ssq = st[..., 1].sum(axis=3)
    var = (ssq - ssum * ssum / N) / (N - 1)
    std = np.sqrt(np.maximum(var, 0.0))
    out_stats = std.mean(axis=(0, 2))            # [tensor] = v, t, pv, pt

    # loss (cols 64..79)
    lo = arr[:, 64:80]
    mean_pt_v = lo[:, 0:8].sum() / N    # phase 0: sum(pt̂ · v̂)
    mean_pv_t = lo[:, 8:16].sum() / N   # phase 1: sum(pv̂ · t̂)
    neg_sim = -0.5 * mean_pv_t - 0.5 * mean_pt_v

    out = np.array([neg_sim,
                    out_stats[0], out_stats[1], out_stats[2], out_stats[3],
                    v_r1, v_r5, v_r10, v_mr,
                    t_r1, t_r5, t_r10, t_mr], dtype=np.float32)
    return out

